# revision 1
# baseline (speedup 1.0000x reference)
"""Embedding lookup + small linear projection on 8 Trainium2 NeuronCores.

Computation (full problem):
    rows = user_repost_matrix[input.reshape(-1)]      # [12800, 2000] f32
    out  = rows @ W.T + b                             # [12800, 8]
    out.reshape(64, 200, 8)

Distribution strategy: pure data-parallel over the 12800 tokens (1600 per
core). The embedding table is replicated into every core's DRAM, so no
collectives are needed: per-core HBM gather traffic (1600 rows x 8KB =
12.8MB) is identical to a row-sharded layout with all-to-all, minus the
communication.

Per-core device kernel (Tile framework), per 128-token tile (13 tiles):
  1. gpsimd.indirect_dma_start gathers 128 table rows -> SBUF R [128, 2000]
  2. PE transposes 16 chunks of [128, 125] f32 -> PSUM [125, 128] (exact)
  3. Split each transposed chunk into bf16 hi + bf16 residual during the
     PSUM->SBUF copies:  RTh = bf16(psum); RTl = bf16(psum - RTh)
  4. Project with two bf16 matmuls per chunk (3-term compensated product,
     ~1e-5 relative error, ~4x cheaper on PE than fp32):
        c[128,16] += RTh^T @ [W2h | W2l]      (hh and h*lo terms)
        c[128,:8] += RTl^T @ W2h              (lo*h term)
  5. C = c[:, :8] + c[:, 8:] + bias on DVE, DMA result slice to DRAM
"""

import sys

if "/opt/trn_rl_repo" not in sys.path:
    sys.path.insert(0, "/opt/trn_rl_repo")

import ml_dtypes
import numpy as np

import concourse.bass as bass
import concourse.tile as tile
from concourse import bacc, mybir
from concourse.bass_utils import run_bass_kernel_spmd
from concourse.masks import make_identity

NTOKEN = 100000
D = 2000
J = 8
B, L = 64, 200
N_CORES = 8
TOK = B * L                      # 12800
PER_CORE = TOK // N_CORES        # 1600
P = 128
TILES = (PER_CORE + P - 1) // P  # 13 (last tile is half-padded)
PAD = TILES * P                  # 1664
KCH = 16                         # feature chunks
KC = D // KCH                    # 125

F32 = mybir.dt.float32
BF16 = mybir.dt.bfloat16
I32 = mybir.dt.int32

_cached = None


def _build():
    """Build + compile the SPMD Bass module once."""
    nc = bacc.Bacc(
        "TRN2", target_bir_lowering=False, debug=False, num_devices=N_CORES
    )
    table = nc.dram_tensor("table", [NTOKEN, D], F32, kind="ExternalInput").ap()
    idx = nc.dram_tensor("idx", [P, TILES], I32, kind="ExternalInput").ap()
    # w2hl[p, k*16 + j]     = bf16(W.T)[k*125 + p, j]          (hi part)
    # w2hl[p, k*16 + 8 + j] = bf16(W.T - hi)[k*125 + p, j]     (lo part)
    w2hl = nc.dram_tensor("w2hl", [KC, KCH * 2 * J], BF16, kind="ExternalInput").ap()
    bias = nc.dram_tensor("bias", [P, J], F32, kind="ExternalInput").ap()
    out = nc.dram_tensor("out", [PAD, J], F32, kind="ExternalOutput").ap()

    with tile.TileContext(nc) as tc:
        with (
            tc.tile_pool(name="const", bufs=1) as cpool,
            tc.tile_pool(name="rows", bufs=4) as rpool,
            tc.tile_pool(name="tpsum", bufs=6, space="PSUM") as tppool,
            tc.tile_pool(name="rth", bufs=6) as rthpool,
            tc.tile_pool(name="rtl", bufs=6) as rtlpool,
            tc.tile_pool(name="cpsum", bufs=2, space="PSUM") as cppool,
            tc.tile_pool(name="o", bufs=2) as opool,
        ):
            idx_sb = cpool.tile([P, TILES], I32)
            nc.sync.dma_start(idx_sb[:], idx[:])
            w2_sb = cpool.tile([KC, KCH * 2 * J], BF16)
            nc.sync.dma_start(w2_sb[:], w2hl[:])
            bias_sb = cpool.tile([P, J], F32)
            nc.sync.dma_start(bias_sb[:], bias[:])
            ident = cpool.tile([P, P], F32)
            make_identity(nc, ident[:])

            for i in range(TILES):
                r = rpool.tile([P, D], F32)
                nc.gpsimd.indirect_dma_start(
                    out=r[:],
                    out_offset=None,
                    in_=table[:],
                    in_offset=bass.IndirectOffsetOnAxis(
                        ap=idx_sb[:, i : i + 1], axis=0
                    ),
                )
                c_ps = cppool.tile([P, 2 * J], F32, space="PSUM")
                for k in range(KCH):
                    t_ps = tppool.tile([KC, P], F32, space="PSUM")
                    nc.tensor.transpose(
                        out=t_ps[:],
                        in_=r[:, k * KC : (k + 1) * KC],
                        identity=ident[:],
                    )
                    rth = rthpool.tile([KC, P], BF16)
                    # round-to-bf16 copy; alternate DVE/ACT (ACT copy is
                    # ~2x DVE, so give ACT only every other one)
                    if k % 2 == 0:
                        nc.scalar.copy(rth[:], t_ps[:])
                    else:
                        nc.vector.tensor_copy(rth[:], t_ps[:])
                    rtl = rtlpool.tile([KC, P], BF16)
                    nc.vector.tensor_tensor(
                        out=rtl[:],
                        in0=t_ps[:],
                        in1=rth[:],
                        op=mybir.AluOpType.subtract,
                    )
                    nc.tensor.matmul(
                        out=c_ps[:],
                        lhsT=rth[:],
                        rhs=w2_sb[:, k * 2 * J : (k + 1) * 2 * J],
                        start=(k == 0),
                        stop=False,
                        skip_group_check=True,
                    )
                    nc.tensor.matmul(
                        out=c_ps[:, :J],
                        lhsT=rtl[:],
                        rhs=w2_sb[:, k * 2 * J : k * 2 * J + J],
                        start=False,
                        stop=(k == KCH - 1),
                        skip_group_check=True,
                    )
                # combine hh + (hl + lh-term) + bias; one PSUM operand per op
                o = opool.tile([P, J], F32)
                nc.vector.tensor_add(o[:], c_ps[:, :J], bias_sb[:])
                nc.vector.tensor_add(o[:], o[:], c_ps[:, J:])
                nc.sync.dma_start(out[i * P : (i + 1) * P, :], o[:])

    nc.compile()
    return nc


def _get_nc():
    global _cached
    if _cached is None:
        _cached = _build()
    return _cached


def _prep_in_maps(input, user_repost_matrix, W, b):
    idx_full = np.asarray(input).reshape(-1).astype(np.int32)
    table = np.ascontiguousarray(np.asarray(user_repost_matrix, dtype=np.float32))
    Wt = np.asarray(W, dtype=np.float32).T                      # [2000, 8]
    # chunked layout: wc[k][p, j] = W.T[k*125+p, j]
    wc = Wt.reshape(KCH, KC, J)                                  # [16, 125, 8]
    wh = wc.astype(ml_dtypes.bfloat16)
    wl = (wc - wh.astype(np.float32)).astype(ml_dtypes.bfloat16)
    # w2hl[p, k, 0:8] = wh[k, p, :]; w2hl[p, k, 8:16] = wl[k, p, :]
    w2hl = np.concatenate([wh, wl], axis=2)                      # [16, 125, 16]
    w2hl = np.ascontiguousarray(
        w2hl.transpose(1, 0, 2).reshape(KC, KCH * 2 * J)
    )
    bias = np.ascontiguousarray(
        np.broadcast_to(np.asarray(b, dtype=np.float32).reshape(1, J), (P, J))
    )
    in_maps = []
    for c in range(N_CORES):
        chunk = idx_full[c * PER_CORE : (c + 1) * PER_CORE]
        padded = np.zeros(PAD, np.int32)
        padded[:PER_CORE] = chunk
        # idx_dram[p, i] = core-local token i*128 + p
        idx_arr = np.ascontiguousarray(padded.reshape(TILES, P).T)
        in_maps.append(
            {"table": table, "idx": idx_arr, "w2hl": w2hl, "bias": bias}
        )
    return in_maps


def _run(in_maps, trace=False, **kw):
    nc = _get_nc()
    return run_bass_kernel_spmd(
        nc, in_maps, list(range(N_CORES)), trace=trace, **kw
    )


def _unshard(results):
    parts = [results[c]["out"][:PER_CORE] for c in range(N_CORES)]
    return np.concatenate(parts, axis=0).reshape(B, L, J).astype(np.float32)


def kernel(input, user_repost_matrix, W, b):
    in_maps = _prep_in_maps(input, user_repost_matrix, W, b)
    res = _run(in_maps)
    return _unshard(res.results)



# revision 5
# speedup vs baseline: 2.0667x; 2.0667x over previous
"""Embedding lookup + small linear projection on 8 Trainium2 NeuronCores.

Computation (full problem):
    rows = user_repost_matrix[input.reshape(-1)]      # [12800, 2000] f32
    out  = rows @ W.T + b                             # [12800, 8]
    out.reshape(64, 200, 8)

Distribution: data-parallel over the 12800 tokens (1600 per core), table
replicated in every core's DRAM (no collectives). The table is staged in
fp16 (rows padded to 2048 elems = 4096B), halving HBM gather traffic; the
dot products are computed in fp16 with fp32 PSUM accumulation (~5e-4 max
rel err, well inside the 2e-2 gate).

Per-core device kernel:
  1. gpsimd.dma_gather(transpose=True) pulls up to 256 table rows per call
     and deposits them TRANSPOSED in SBUF as [128, 16, ntok] fp16 --
     feature f = k*128 + p lands on partition p, chunk k. This removes the
     PE transpose + PSUM round-trip of the previous design entirely.
  2. Per 16 feature-chunks: one fp16 matmul psum[8, ntok] += W_k^T @ rows_k
     (W chunk [128, 8] stationary, gathered tokens moving).
  3. DVE adds bias (per-partition scalar) while copying PSUM -> SBUF f32,
     DMA result slice to DRAM out[8, TOT]; host transposes/unpermutes.

dma_gather indices are int16 (< 32768), so the 100000-row table is split
into 4 base-offset groups of 25000 rows. Tokens are grouped by row-group
on the host, balanced across cores so every core has identical per-group
counts (global pad to a multiple of 8 with dummy index-0 tokens), and each
group is gathered from its own table base AP. Trailing -1 indices pad each
group to a 128-multiple; the gather ucode only transfers up to
roundup(valid, 16) rows, and garbage columns only pollute their own output
column (matmul columns are independent), which the host drops.
"""

import sys

if "/opt/trn_rl_repo" not in sys.path:
    sys.path.insert(0, "/opt/trn_rl_repo")

import ml_dtypes
import numpy as np

import concourse.bass as bass
import concourse.tile as tile
from concourse import bacc, mybir
from concourse.bass_utils import run_bass_kernel_spmd

NTOKEN = 100000
D = 2000
DPAD = 2048                      # fp16 row padded to 4096 bytes
J = 8
B, L = 64, 200
N_CORES = 8
TOK = B * L                      # 12800
PER_CORE = TOK // N_CORES        # 1600
NGROUPS = 4
GR = 25000                       # table rows per index group (fits int16)
KCH = DPAD // 128                # 16 feature chunks of 128
SUB = 256                        # tokens per gather / matmul subtile

F32 = mybir.dt.float32
FP16 = mybir.dt.float16
I16 = mybir.dt.int16

_cached = {}


def _roundup(x, m):
    return (x + m - 1) // m * m


def _subtiles(n_gs):
    """Static subtile schedule: (group, global col off, size, valid)."""
    subs = []
    off = 0
    for g in range(NGROUPS):
        n = n_gs[g]
        cap = _roundup(max(n, 1), 128)
        start = 0
        while start < cap:
            sz = min(SUB, cap - start)
            valid = min(n, start + sz) - start
            subs.append((g, off + start, sz, valid))
            start += sz
        off += cap
    return subs, off


def _build(n_gs):
    """Build + compile the SPMD Bass module for per-core group sizes n_gs."""
    subs, tot = _subtiles(n_gs)
    nc = bacc.Bacc(
        "TRN2", target_bir_lowering=False, debug=False, num_devices=N_CORES
    )
    table = nc.dram_tensor("table", [NTOKEN, DPAD], FP16, kind="ExternalInput").ap()
    # [128, n/16]: token i of a gather window at [i % 16, i // 16], with the
    # 16-partition block replicated for each of the 8 Q7 cores.
    idxs = nc.dram_tensor("idxs", [128, tot // 16], I16, kind="ExternalInput").ap()
    wmat = nc.dram_tensor("w", [128, KCH * J], FP16, kind="ExternalInput").ap()
    bias = nc.dram_tensor("bias", [J, 1], F32, kind="ExternalInput").ap()
    out = nc.dram_tensor("out", [J, tot], F32, kind="ExternalOutput").ap()

    with tile.TileContext(nc) as tc:
        with (
            tc.tile_pool(name="const", bufs=1) as cpool,
            tc.tile_pool(name="gath", bufs=4) as gpool,
            tc.tile_pool(name="acc", bufs=4, space="PSUM") as ppool,
            tc.tile_pool(name="o", bufs=3) as opool,
        ):
            idx_sb = cpool.tile([128, tot // 16], I16)
            nc.gpsimd.dma_start(idx_sb[:], idxs[:])
            w_sb = cpool.tile([128, KCH * J], FP16)
            nc.sync.dma_start(w_sb[:], wmat[:])
            bias_sb = cpool.tile([J, 1], F32)
            nc.sync.dma_start(bias_sb[:], bias[:])

            for g, coff, sz, valid in subs:
                gt = gpool.tile([128, KCH, sz], FP16)
                nc.gpsimd.dma_gather(
                    gt[:],
                    table[g * GR : (g + 1) * GR, :],
                    idx_sb[:, coff // 16 : (coff + sz) // 16],
                    sz,
                    valid,
                    DPAD,
                    transpose=True,
                )
                ps = ppool.tile([J, sz], F32, space="PSUM")
                for k in range(KCH):
                    nc.tensor.matmul(
                        out=ps[:],
                        lhsT=w_sb[:, k * J : (k + 1) * J],
                        rhs=gt[:, k, :],
                        start=(k == 0),
                        stop=(k == KCH - 1),
                    )
                ot = opool.tile([J, sz], F32)
                nc.vector.tensor_scalar_add(
                    ot[:, :valid], ps[:, :valid], bias_sb[:, 0:1]
                )
                nc.sync.dma_start(out[:, coff : coff + valid], ot[:, :valid])

    nc.compile()
    return nc


def _get_nc(n_gs):
    key = tuple(n_gs)
    if key not in _cached:
        _cached[key] = _build(key)
    return _cached[key]


def _prep_in_maps(input, user_repost_matrix, W, b):
    idx_full = np.asarray(input).reshape(-1).astype(np.int64)
    assert idx_full.shape[0] == TOK

    # Partition tokens by table row-group, balanced across cores.
    grp = (idx_full // GR).astype(np.int64)
    # core_tok[c][g] -> (local_idx int16 array, orig_pos int64 array)
    core_tok = [[None] * NGROUPS for _ in range(N_CORES)]
    n_gs = []
    for g in range(NGROUPS):
        pos = np.nonzero(grp == g)[0]
        # pad globally to a multiple of N_CORES with dummy tokens (row 0 of
        # this group, orig position -1)
        npad = _roundup(len(pos), N_CORES) - len(pos)
        loc = (idx_full[pos] - g * GR).astype(np.int16)
        if npad:
            loc = np.concatenate([loc, np.zeros(npad, np.int16)])
            pos = np.concatenate([pos, np.full(npad, -1, np.int64)])
        n_gs.append(len(pos) // N_CORES)
        for c in range(N_CORES):
            core_tok[c][g] = (loc[c::N_CORES], pos[c::N_CORES])
    n_gs = tuple(n_gs)
    subs, tot = _subtiles(n_gs)

    table16 = np.zeros((NTOKEN, DPAD), dtype=np.float16)
    table16[:, :D] = np.asarray(user_repost_matrix, dtype=np.float32)

    # w_sb[p, k*8 + j] = W.T[k*128 + p, j]
    wt = np.zeros((DPAD, J), dtype=np.float16)
    wt[:D] = np.asarray(W, dtype=np.float32).T
    w_sb = np.ascontiguousarray(
        wt.reshape(KCH, 128, J).transpose(1, 0, 2).reshape(128, KCH * J)
    )
    bias_arr = np.ascontiguousarray(
        np.asarray(b, dtype=np.float32).reshape(J, 1)
    )

    in_maps = []
    pos_flat_all = []
    for c in range(N_CORES):
        idx_flat = np.full(tot, -1, np.int16)
        pos_flat = np.full(tot, -1, np.int64)
        off = 0
        for g in range(NGROUPS):
            loc, pos = core_tok[c][g]
            n = n_gs[g]
            idx_flat[off : off + n] = loc
            pos_flat[off : off + n] = pos
            off += _roundup(max(n, 1), 128)
        assert off == tot
        # idx_dram[r, col]: flat token p lives at [p % 16, p // 16]; the
        # 16-row block is tiled 8x down the partition dim (one copy per
        # Q7 core).
        idx_arr = np.ascontiguousarray(
            np.tile(idx_flat.reshape(tot // 16, 16).T, (N_CORES, 1))
        )
        pos_flat_all.append(pos_flat)
        in_maps.append(
            {"table": table16, "idxs": idx_arr, "w": w_sb, "bias": bias_arr}
        )
    return in_maps, n_gs, pos_flat_all


def _run(in_maps, n_gs, trace=False, **kw):
    nc = _get_nc(n_gs)
    return run_bass_kernel_spmd(
        nc, in_maps, list(range(N_CORES)), trace=trace, **kw
    )


def _unshard(results, pos_flat_all):
    full = np.empty((TOK, J), dtype=np.float32)
    for c in range(N_CORES):
        res = results[c]["out"]                     # [8, tot] f32
        pos = pos_flat_all[c]
        valid = pos >= 0
        full[pos[valid]] = res.T[valid]
    return full.reshape(B, L, J)


def kernel(input, user_repost_matrix, W, b):
    in_maps, n_gs, pos_all = _prep_in_maps(input, user_repost_matrix, W, b)
    res = _run(in_maps, n_gs)
    return _unshard(res.results, pos_all)


# revision 11
# speedup vs baseline: 2.0965x; 1.0144x over previous
"""Embedding lookup + small linear projection on 8 Trainium2 NeuronCores.

Computation (full problem):
    rows = user_repost_matrix[input.reshape(-1)]      # [12800, 2000] f32
    out  = rows @ W.T + b                             # [12800, 8]
    out.reshape(64, 200, 8)

Distribution: data-parallel over the 12800 tokens (1600 per core), table
replicated in every core's DRAM (no collectives). The table is staged in
fp16 (rows padded to 2048 elems = 4096B), halving HBM gather traffic; the
dot products are computed in fp16 with fp32 PSUM accumulation (~5e-4 max
rel err, well inside the 2e-2 gate).

Per-core device kernel:
  1. gpsimd.dma_gather(transpose=True) pulls up to 256 table rows per call
     and deposits them TRANSPOSED in SBUF as [128, 16, ntok] fp16 --
     feature f = k*128 + p lands on partition p, chunk k. This removes the
     PE transpose + PSUM round-trip of the previous design entirely.
  2. Per 16 feature-chunks: one fp16 matmul psum[8, ntok] += W_k^T @ rows_k
     (W chunk [128, 8] stationary, gathered tokens moving).
  3. DVE adds bias (per-partition scalar) while copying PSUM -> SBUF f32,
     DMA result slice to DRAM out[8, TOT]; host transposes/unpermutes.

dma_gather indices are int16 (< 32768), so the 100000-row table is split
into 4 base-offset groups of 25000 rows. Tokens are grouped by row-group
on the host, balanced across cores so every core has identical per-group
counts (global pad to a multiple of 8 with dummy index-0 tokens), and each
group is gathered from its own table base AP. Trailing -1 indices pad each
group to a 128-multiple; the gather ucode only transfers up to
roundup(valid, 16) rows, and garbage columns only pollute their own output
column (matmul columns are independent), which the host drops.
"""

import sys

if "/opt/trn_rl_repo" not in sys.path:
    sys.path.insert(0, "/opt/trn_rl_repo")

import ml_dtypes
import numpy as np

import concourse.bass as bass
import concourse.tile as tile
from concourse import bacc, library_config, mybir
from concourse.bass_utils import run_bass_kernel_spmd

NTOKEN = 100000
D = 2000
DPAD = 2048                      # fp16 row padded to 4096 bytes
J = 8
B, L = 64, 200
N_CORES = 8
TOK = B * L                      # 12800
PER_CORE = TOK // N_CORES        # 1600
NGROUPS = 4
GR = 25000                       # table rows per index group (fits int16)
KCH = DPAD // 128                # 16 feature chunks of 128
SUB = 256                        # tokens per gather / matmul subtile

F32 = mybir.dt.float32
FP16 = mybir.dt.float16
I16 = mybir.dt.int16

_cached = {}


def _roundup(x, m):
    return (x + m - 1) // m * m


def _subtiles(n_gs):
    """Static subtile schedule: (group, global col off, size, valid)."""
    subs = []
    off = 0
    for g in range(NGROUPS):
        n = n_gs[g]
        cap = _roundup(max(n, 1), 128)
        start = 0
        while start < cap:
            sz = min(SUB, cap - start)
            valid = min(n, start + sz) - start
            subs.append((g, off + start, sz, valid))
            start += sz
        off += cap
    return subs, off


def _build(n_gs):
    """Build + compile the SPMD Bass module for per-core group sizes n_gs."""
    subs, tot = _subtiles(n_gs)
    nc = bacc.Bacc(
        "TRN2", target_bir_lowering=False, debug=False, num_devices=N_CORES
    )
    table = nc.dram_tensor("table", [NTOKEN, DPAD], FP16, kind="ExternalInput").ap()
    # [128, n/16]: token i of a gather window at [i % 16, i // 16], with the
    # 16-partition block replicated for each of the 8 Q7 cores.
    idxs = nc.dram_tensor("idxs", [128, tot // 16], I16, kind="ExternalInput").ap()
    wmat = nc.dram_tensor("w", [128, KCH * J], FP16, kind="ExternalInput").ap()
    bias = nc.dram_tensor("bias", [J, 1], F32, kind="ExternalInput").ap()
    out = nc.dram_tensor("out", [J, tot], F32, kind="ExternalOutput").ap()

    with tile.TileContext(nc) as tc:
        with (
            tc.tile_pool(name="const", bufs=1) as cpool,
            tc.tile_pool(name="gath", bufs=8) as gpool,
            tc.tile_pool(name="acc", bufs=4, space="PSUM") as ppool,
        ):
            # Get the Q7 ucode reload going before anything else on Pool;
            # the gather ucode load is ~8us and everything waits on it.
            nc.gpsimd.load_library(library_config.mlp)
            idx_sb = cpool.tile([128, tot // 16], I16)
            nc.sync.dma_start(idx_sb[:], idxs[:])
            w_sb = cpool.tile([128, KCH * J], FP16)
            nc.sync.dma_start(w_sb[:], wmat[:])
            bias_sb = cpool.tile([J, 1], F32)
            nc.sync.dma_start(bias_sb[:], bias[:])
            out_sb = cpool.tile([J, tot], F32)

            for i, (g, coff, sz, valid) in enumerate(subs):
                gt = gpool.tile([128, KCH, sz], FP16)
                nc.gpsimd.dma_gather(
                    gt[:],
                    table[g * GR : (g + 1) * GR, :],
                    idx_sb[:, coff // 16 : (coff + sz) // 16],
                    sz,
                    valid,
                    DPAD,
                    transpose=True,
                )
                ps = ppool.tile([J, sz], F32, space="PSUM")
                for k in range(KCH):
                    nc.tensor.matmul(
                        out=ps[:],
                        lhsT=w_sb[:, k * J : (k + 1) * J],
                        rhs=gt[:, k, :],
                        start=(k == 0),
                        stop=(k == KCH - 1),
                    )
                nc.vector.tensor_scalar_add(
                    out_sb[:, coff : coff + valid],
                    ps[:, :valid],
                    bias_sb[:, 0:1],
                )
            nc.sync.dma_start(out[:], out_sb[:])

    nc.compile()
    return nc


def _get_nc(n_gs):
    key = tuple(n_gs)
    if key not in _cached:
        _cached[key] = _build(key)
    return _cached[key]


def _prep_in_maps(input, user_repost_matrix, W, b):
    idx_full = np.asarray(input).reshape(-1).astype(np.int64)
    assert idx_full.shape[0] == TOK

    # Partition tokens by table row-group, balanced across cores.
    grp = (idx_full // GR).astype(np.int64)
    # core_tok[c][g] -> (local_idx int16 array, orig_pos int64 array)
    core_tok = [[None] * NGROUPS for _ in range(N_CORES)]
    n_gs = []
    for g in range(NGROUPS):
        pos = np.nonzero(grp == g)[0]
        # pad globally to a multiple of N_CORES with dummy tokens (row 0 of
        # this group, orig position -1); keep at least one real slot per
        # core so no gather ends up with zero valid indices
        npad = _roundup(max(len(pos), 1), N_CORES) - len(pos)
        loc = (idx_full[pos] - g * GR).astype(np.int16)
        if npad:
            loc = np.concatenate([loc, np.zeros(npad, np.int16)])
            pos = np.concatenate([pos, np.full(npad, -1, np.int64)])
        n_gs.append(len(pos) // N_CORES)
        for c in range(N_CORES):
            core_tok[c][g] = (loc[c::N_CORES], pos[c::N_CORES])
    n_gs = tuple(n_gs)
    subs, tot = _subtiles(n_gs)

    table16 = np.zeros((NTOKEN, DPAD), dtype=np.float16)
    table16[:, :D] = np.asarray(user_repost_matrix, dtype=np.float32)

    # w_sb[p, k*8 + j] = W.T[k*128 + p, j]
    wt = np.zeros((DPAD, J), dtype=np.float16)
    wt[:D] = np.asarray(W, dtype=np.float32).T
    w_sb = np.ascontiguousarray(
        wt.reshape(KCH, 128, J).transpose(1, 0, 2).reshape(128, KCH * J)
    )
    bias_arr = np.ascontiguousarray(
        np.asarray(b, dtype=np.float32).reshape(J, 1)
    )

    in_maps = []
    pos_flat_all = []
    for c in range(N_CORES):
        idx_flat = np.full(tot, -1, np.int16)
        pos_flat = np.full(tot, -1, np.int64)
        off = 0
        for g in range(NGROUPS):
            loc, pos = core_tok[c][g]
            n = n_gs[g]
            idx_flat[off : off + n] = loc
            pos_flat[off : off + n] = pos
            off += _roundup(max(n, 1), 128)
        assert off == tot
        # idx_dram[r, col]: flat token p lives at [p % 16, p // 16]; the
        # 16-row block is tiled 8x down the partition dim (one copy per
        # Q7 core).
        idx_arr = np.ascontiguousarray(
            np.tile(idx_flat.reshape(tot // 16, 16).T, (N_CORES, 1))
        )
        pos_flat_all.append(pos_flat)
        in_maps.append(
            {"table": table16, "idxs": idx_arr, "w": w_sb, "bias": bias_arr}
        )
    return in_maps, n_gs, pos_flat_all


def _run(in_maps, n_gs, trace=False, **kw):
    nc = _get_nc(n_gs)
    return run_bass_kernel_spmd(
        nc, in_maps, list(range(N_CORES)), trace=trace, **kw
    )


def _unshard(results, pos_flat_all):
    full = np.empty((TOK, J), dtype=np.float32)
    for c in range(N_CORES):
        res = results[c]["out"]                     # [8, tot] f32
        pos = pos_flat_all[c]
        valid = pos >= 0
        full[pos[valid]] = res.T[valid]
    return full.reshape(B, L, J)


def kernel(input, user_repost_matrix, W, b):
    in_maps, n_gs, pos_all = _prep_in_maps(input, user_repost_matrix, W, b)
    res = _run(in_maps, n_gs)
    return _unshard(res.results, pos_all)
